# revision 6
# baseline (speedup 1.0000x reference)
"""GRACE contrastive loss on 8 Trainium2 NeuronCores (Bass/Tile).

loss = mean over i of 0.5*(l1_i + l2_i), where (T=0.5, a/b = row-normalized
h1/h2):
  l1_i = log(sum_j exp(a_i.a_j/T) - e^2 + sum_j exp(a_i.b_j/T)) - a_i.b_i/T
  l2_i = log(sum_j exp(b_i.b_j/T) - e^2 + sum_j exp(b_i.a_j/T)) - a_i.b_i/T

Sharding: rows of the NxN similarity matrices are split over 8 cores
(1024 rows each). Each core computes three 1024x8192 matmul+exp slabs
(a_blk@a^T, b_blk@b^T, a_blk@b^T) with fused row-sum accumulation on the
scalar engine, plus column sums of exp(a_blk@b^T) via a ones-vector
matmul on the tensor engine (these partial column sums are added across
cores on the host to give sum_j exp(b_i.a_j/T)). The host does the cheap
O(N*D) pieces: row normalization, the diagonal a_i.b_i, final log/mean.
"""

import hashlib
import os
import pickle
from contextlib import ExitStack
from pathlib import Path

import ml_dtypes
import numpy as np

TEMPERATURE = 0.5
EPS = 1e-8
N, D = 8192, 128
NCORES = 8
BLK = N // NCORES          # rows per core
RT = BLK // 128            # 128-row tiles per core block
# ACT chunk layout per row-tile: 5x1536 + 512 = 8192 columns
CHUNKS = [(0, 1536), (1536, 1536), (3072, 1536), (4608, 1536), (6144, 1536), (7680, 512)]


def _install_neff_disk_cache():
    """Cache walrus NEFF compiles on disk so fresh-process runs are fast."""
    import concourse.bass2jax as bass2jax

    if getattr(bass2jax, "_grace_neff_cache", False):
        return
    inner = bass2jax.compile_bir_kernel
    cache_dir = Path(os.environ.get("XDG_CACHE_HOME", os.path.expanduser("~/.cache")))
    cache_dir = cache_dir / "bass_neff_cache"
    try:
        cache_dir.mkdir(parents=True, exist_ok=True)
    except OSError:
        return

    def cached(bir_json, tmpdir, neff_name="file.neff"):
        data = bir_json if isinstance(bir_json, bytes) else bir_json.encode()
        key = hashlib.sha256(data).hexdigest()
        path = cache_dir / f"{key}_{neff_name}"
        out_path = os.path.join(tmpdir, neff_name)
        if path.exists():
            with open(path, "rb") as f:
                neff = f.read()
            with open(out_path, "wb") as f:
                f.write(neff)
            return out_path
        res = inner(bir_json, tmpdir, neff_name)
        try:
            with open(res, "rb") as f:
                neff = f.read()
            tmp = path.with_suffix(".tmp%d" % os.getpid())
            with open(tmp, "wb") as f:
                f.write(neff)
            tmp.rename(path)
        except OSError:
            pass
        return res

    bass2jax.compile_bir_kernel = cached
    bass2jax._grace_neff_cache = True


_PROGRAM = None


def build_program():
    """Build (once) the per-core Bass program. Identical on all 8 cores;
    per-core data (the core's row block) arrives as separate inputs."""
    global _PROGRAM
    if _PROGRAM is not None:
        return _PROGRAM

    import concourse.tile as tile
    from concourse import bacc, mybir

    BF = mybir.dt.bfloat16
    F32 = mybir.dt.float32
    Exp = mybir.ActivationFunctionType.Exp
    X = mybir.AxisListType.X

    nc = bacc.Bacc(
        "TRN2",
        target_bir_lowering=False,
        debug=False,
        enable_asserts=False,
        num_devices=NCORES,
    )
    at_d = nc.dram_tensor("at", [128, N], BF, kind="ExternalInput").ap()
    bt_d = nc.dram_tensor("bt", [128, N], BF, kind="ExternalInput").ap()
    abt_d = nc.dram_tensor("abt", [128, BLK], BF, kind="ExternalInput").ap()
    bbt_d = nc.dram_tensor("bbt", [128, BLK], BF, kind="ExternalInput").ap()
    rs_ab_d = nc.dram_tensor("rs_ab", [128, RT], F32, kind="ExternalOutput").ap()
    rs_aa_d = nc.dram_tensor("rs_aa", [128, RT], F32, kind="ExternalOutput").ap()
    rs_bb_d = nc.dram_tensor("rs_bb", [128, RT], F32, kind="ExternalOutput").ap()
    cs_ab_d = nc.dram_tensor("cs_ab", [1, N], BF, kind="ExternalOutput").ap()

    with tile.TileContext(nc) as tc, ExitStack() as ctx:
        inp = ctx.enter_context(tc.tile_pool(name="inp", bufs=1))
        expp = ctx.enter_context(tc.tile_pool(name="expst", bufs=1))
        accp = ctx.enter_context(tc.tile_pool(name="acc", bufs=4))
        rsp = ctx.enter_context(tc.tile_pool(name="rs", bufs=3))
        onep = ctx.enter_context(tc.tile_pool(name="ones", bufs=1))
        csbp = ctx.enter_context(tc.tile_pool(name="csb", bufs=1))
        mmp = ctx.enter_context(tc.tile_pool(name="mm", bufs=2, space="PSUM"))
        csp = ctx.enter_context(tc.tile_pool(name="cs", bufs=2, space="PSUM"))

        # Input DMAs, in first-use order (betw slab runs first).
        abt = inp.tile([128, BLK], BF)
        nc.sync.dma_start(abt[:], abt_d[:])
        bt = inp.tile([128, N], BF)
        nc.sync.dma_start(bt[:], bt_d[:])
        at = inp.tile([128, N], BF)
        nc.sync.dma_start(at[:], at_d[:])
        bbt = inp.tile([128, BLK], BF)
        nc.sync.dma_start(bbt[:], bbt_d[:])

        ones = onep.tile([128, 1], BF)
        nc.vector.memset(ones[:], 1.0)
        # Staging for exp(a_blk@b^T): RT row-tiles of [128, N] in bf16.
        expst = expp.tile([128, RT * N], BF)

        def slab(lhs_sb, rhs_sb, rs_dram, store_exp):
            rs_t = rsp.tile([128, RT], F32)
            for rt in range(RT):
                lhsT = lhs_sb[:, rt * 128 : (rt + 1) * 128]
                acc = accp.tile([128, len(CHUNKS)], F32)
                for ci, (off, sz) in enumerate(CHUNKS):
                    mt = mmp.tile([128, 1536], F32)
                    for q in range(sz // 512):
                        nc.tensor.matmul(
                            mt[:, q * 512 : (q + 1) * 512],
                            lhsT=lhsT,
                            rhs=rhs_sb[:, off + q * 512 : off + (q + 1) * 512],
                            start=True,
                            stop=True,
                        )
                    if store_exp:
                        o = expst[:, rt * N + off : rt * N + off + sz]
                    else:
                        o = mt[:, :sz]  # in-place over PSUM; values discarded
                    nc.scalar.activation(
                        o, mt[:, :sz], Exp, scale=2.0, accum_out=acc[:, ci : ci + 1]
                    )
                nc.vector.reduce_sum(rs_t[:, rt : rt + 1], acc[:, :], axis=X)
            nc.sync.dma_start(rs_dram[:], rs_t[:])

        # 1) between slab (stores exp for column sums)
        slab(abt, bt, rs_ab_d, True)
        # 2) column sums of the between slab: ones^T @ exp tiles, PSUM-accumulated
        #    across the RT row tiles. Overlaps with the refl slabs' ACT work.
        cs_sb = csbp.tile([1, N], BF)
        for ct in range(N // 512):
            cst = csp.tile([1, 512], F32)
            for rt in range(RT):
                nc.tensor.matmul(
                    cst[:, :],
                    lhsT=ones[:, :],
                    rhs=expst[:, rt * N + ct * 512 : rt * N + (ct + 1) * 512],
                    start=(rt == 0),
                    stop=(rt == RT - 1),
                )
            nc.vector.tensor_copy(cs_sb[:, ct * 512 : (ct + 1) * 512], cst[:, :])
        nc.sync.dma_start(cs_ab_d[:], cs_sb[:])
        # 3) reflexive slabs (row sums only)
        slab(abt, at, rs_aa_d, False)
        slab(bbt, bt, rs_bb_d, False)

    nc.compile()
    _PROGRAM = nc
    return nc


def _normalize(x):
    n = np.linalg.norm(x, axis=1, keepdims=True)
    return x / np.maximum(n, EPS)


def kernel(h1: np.ndarray, h2: np.ndarray):
    h1 = np.asarray(h1, dtype=np.float32)
    h2 = np.asarray(h2, dtype=np.float32)
    assert h1.shape == (N, D) and h2.shape == (N, D)

    a = _normalize(h1)
    b = _normalize(h2)
    diag = np.einsum("ij,ij->i", a, b, dtype=np.float64)

    bf = ml_dtypes.bfloat16
    at = np.ascontiguousarray(a.T).astype(bf)   # [128, 8192]
    bt = np.ascontiguousarray(b.T).astype(bf)

    _install_neff_disk_cache()
    nc = build_program()

    from concourse import bass_utils

    in_maps = []
    for c in range(NCORES):
        s = slice(c * BLK, (c + 1) * BLK)
        in_maps.append(
            {
                "at": at,
                "bt": bt,
                "abt": np.ascontiguousarray(at[:, s]),
                "bbt": np.ascontiguousarray(bt[:, s]),
            }
        )

    res = bass_utils.run_bass_kernel_spmd(nc, in_maps, core_ids=list(range(NCORES)))

    # Assemble per-row sums. rs tiles are [128 partitions, RT row-tiles]:
    # global row index = core*BLK + rt*128 + p  ->  rs.T.reshape(BLK).
    def rows(name):
        return np.concatenate(
            [res.results[c][name].astype(np.float64).T.reshape(BLK) for c in range(NCORES)]
        )

    rs_ab = rows("rs_ab")
    rs_aa = rows("rs_aa")
    rs_bb = rows("rs_bb")
    cs_ab = np.sum(
        [res.results[c]["cs_ab"][0].astype(np.float64) for c in range(NCORES)], axis=0
    )

    e2 = np.exp(2.0)
    denom1 = rs_aa - e2 + rs_ab
    denom2 = rs_bb - e2 + cs_ab
    l1 = np.log(denom1) - 2.0 * diag
    l2 = np.log(denom2) - 2.0 * diag
    loss = np.mean(0.5 * (l1 + l2))
    return (np.asarray(loss, dtype=np.float32), 1)


# revision 8
# speedup vs baseline: 1.2563x; 1.2563x over previous
"""GRACE contrastive loss on 8 Trainium2 NeuronCores (Bass/Tile).

loss = mean over i of 0.5*(l1_i + l2_i), where (T=0.5, a/b = row-normalized
h1/h2):
  l1_i = log(sum_j exp(a_i.a_j/T) - e^2 + sum_j exp(a_i.b_j/T)) - a_i.b_i/T
  l2_i = log(sum_j exp(b_i.b_j/T) - e^2 + sum_j exp(b_i.a_j/T)) - a_i.b_i/T

Sharding: rows of the NxN similarity matrices are split over 8 cores
(1024 rows each). Each core computes three 1024x8192 matmul+exp slabs
(a_blk@a^T, b_blk@b^T, a_blk@b^T) with fused row-sum accumulation on the
scalar engine, plus column sums of exp(a_blk@b^T) via a ones-vector
matmul on the tensor engine (these partial column sums are added across
cores on the host to give sum_j exp(b_i.a_j/T)). The host does the cheap
O(N*D) pieces: row normalization, the diagonal a_i.b_i, final log/mean.
"""

import hashlib
import os
import pickle
from contextlib import ExitStack
from pathlib import Path

import ml_dtypes
import numpy as np

TEMPERATURE = 0.5
EPS = 1e-8
N, D = 8192, 128
NCORES = 8
BLK = N // NCORES          # rows per core
RT = BLK // 128            # 128-row tiles per core block
# ACT chunk layout per row-tile: 5x1536 + 512 = 8192 columns
CHUNKS = [(0, 1536), (1536, 1536), (3072, 1536), (4608, 1536), (6144, 1536), (7680, 512)]


def _install_neff_disk_cache():
    """Cache walrus NEFF compiles on disk so fresh-process runs are fast."""
    import concourse.bass2jax as bass2jax

    if getattr(bass2jax, "_grace_neff_cache", False):
        return
    inner = bass2jax.compile_bir_kernel
    cache_dir = Path(os.environ.get("XDG_CACHE_HOME", os.path.expanduser("~/.cache")))
    cache_dir = cache_dir / "bass_neff_cache"
    try:
        cache_dir.mkdir(parents=True, exist_ok=True)
    except OSError:
        return

    def cached(bir_json, tmpdir, neff_name="file.neff"):
        data = bir_json if isinstance(bir_json, bytes) else bir_json.encode()
        key = hashlib.sha256(data).hexdigest()
        path = cache_dir / f"{key}_{neff_name}"
        out_path = os.path.join(tmpdir, neff_name)
        if path.exists():
            with open(path, "rb") as f:
                neff = f.read()
            with open(out_path, "wb") as f:
                f.write(neff)
            return out_path
        res = inner(bir_json, tmpdir, neff_name)
        try:
            with open(res, "rb") as f:
                neff = f.read()
            tmp = path.with_suffix(".tmp%d" % os.getpid())
            with open(tmp, "wb") as f:
                f.write(neff)
            tmp.rename(path)
        except OSError:
            pass
        return res

    bass2jax.compile_bir_kernel = cached
    bass2jax._grace_neff_cache = True


_PROGRAM = None


def build_program():
    """Build (once) the per-core Bass program. Identical on all 8 cores;
    per-core data (the core's row block) arrives as separate inputs."""
    global _PROGRAM
    if _PROGRAM is not None:
        return _PROGRAM

    import concourse.tile as tile
    from concourse import bacc, mybir

    BF = mybir.dt.bfloat16
    F32 = mybir.dt.float32
    Exp = mybir.ActivationFunctionType.Exp
    X = mybir.AxisListType.X

    nc = bacc.Bacc(
        "TRN2",
        target_bir_lowering=False,
        debug=False,
        enable_asserts=False,
        num_devices=NCORES,
    )
    at_d = nc.dram_tensor("at", [128, N], BF, kind="ExternalInput").ap()
    bt_d = nc.dram_tensor("bt", [128, N], BF, kind="ExternalInput").ap()
    abt_d = nc.dram_tensor("abt", [128, BLK], BF, kind="ExternalInput").ap()
    bbt_d = nc.dram_tensor("bbt", [128, BLK], BF, kind="ExternalInput").ap()
    rs_ab_d = nc.dram_tensor("rs_ab", [128, RT], F32, kind="ExternalOutput").ap()
    rs_aa_d = nc.dram_tensor("rs_aa", [128, RT], F32, kind="ExternalOutput").ap()
    rs_bb_d = nc.dram_tensor("rs_bb", [128, RT], F32, kind="ExternalOutput").ap()
    cs_ab_d = nc.dram_tensor("cs_ab", [1, N], BF, kind="ExternalOutput").ap()

    with tile.TileContext(nc) as tc, ExitStack() as ctx:
        inp = ctx.enter_context(tc.tile_pool(name="inp", bufs=1))
        expp = ctx.enter_context(tc.tile_pool(name="expst", bufs=1))
        accp = ctx.enter_context(tc.tile_pool(name="acc", bufs=4))
        rsp = ctx.enter_context(tc.tile_pool(name="rs", bufs=3))
        onep = ctx.enter_context(tc.tile_pool(name="ones", bufs=1))
        csbp = ctx.enter_context(tc.tile_pool(name="csb", bufs=1))
        mmp = ctx.enter_context(tc.tile_pool(name="mm", bufs=2, space="PSUM"))
        csp = ctx.enter_context(tc.tile_pool(name="cs", bufs=2, space="PSUM"))

        # Input DMAs, in first-use order (betw slab runs first). at/bt are
        # split into 4 pieces so the first matmuls start after ~0.5MB.
        PIECE = N // 4
        abt = inp.tile([128, BLK], BF)
        nc.sync.dma_start(abt[:], abt_d[:])
        bt_p = []
        at_p = []
        for i in range(4):
            t = inp.tile([128, PIECE], BF, tag=f"bt{i}")
            nc.sync.dma_start(t[:], bt_d[:, i * PIECE : (i + 1) * PIECE])
            bt_p.append(t)
        for i in range(4):
            t = inp.tile([128, PIECE], BF, tag=f"at{i}")
            nc.sync.dma_start(t[:], at_d[:, i * PIECE : (i + 1) * PIECE])
            at_p.append(t)
        bbt = inp.tile([128, BLK], BF)
        nc.sync.dma_start(bbt[:], bbt_d[:])

        def rhs_slice(pieces, off, sz):
            # [off, off+sz) must lie inside one piece
            pi, po = off // PIECE, off % PIECE
            assert po + sz <= PIECE
            return pieces[pi][:, po : po + sz]

        ones = onep.tile([128, 1], BF)
        nc.vector.memset(ones[:], 1.0)
        # Staging for exp(a_blk@b^T): RT row-tiles of [128, N] in bf16.
        expst = expp.tile([128, RT * N], BF)

        cs_sb = csbp.tile([1, N], BF)

        def cs_group(ct):
            # column sums of exp(a_blk@b^T) for columns [ct*512, (ct+1)*512):
            # ones^T @ exp tiles, PSUM-accumulated across the RT row tiles.
            cst = csp.tile([1, 512], F32)
            for rt in range(RT):
                nc.tensor.matmul(
                    cst[:, :],
                    lhsT=ones[:, :],
                    rhs=expst[:, rt * N + ct * 512 : rt * N + (ct + 1) * 512],
                    start=(rt == 0),
                    stop=(rt == RT - 1),
                )
            nc.vector.tensor_copy(cs_sb[:, ct * 512 : (ct + 1) * 512], cst[:, :])

        def slab(lhs_sb, rhs_pieces, rs_dram, store_exp, after_row=None):
            rs_t = rsp.tile([128, RT], F32)
            for rt in range(RT):
                lhsT = lhs_sb[:, rt * 128 : (rt + 1) * 128]
                acc = accp.tile([128, len(CHUNKS)], F32)
                for ci, (off, sz) in enumerate(CHUNKS):
                    mt = mmp.tile([128, 1536], F32)
                    for q in range(sz // 512):
                        nc.tensor.matmul(
                            mt[:, q * 512 : (q + 1) * 512],
                            lhsT=lhsT,
                            rhs=rhs_slice(rhs_pieces, off + q * 512, 512),
                            start=True,
                            stop=True,
                        )
                    if store_exp:
                        o = expst[:, rt * N + off : rt * N + off + sz]
                    else:
                        o = mt[:, :sz]  # in-place over PSUM; values discarded
                    nc.scalar.activation(
                        o, mt[:, :sz], Exp, scale=2.0, accum_out=acc[:, ci : ci + 1]
                    )
                nc.vector.reduce_sum(rs_t[:, rt : rt + 1], acc[:, :], axis=X)
                if after_row is not None:
                    after_row(rt)
            nc.sync.dma_start(rs_dram[:], rs_t[:])

        # 1) between slab (stores exp for column sums)
        slab(abt, bt_p, rs_ab_d, True)
        # 2) reflexive slabs (row sums only), with one cs group interleaved
        #    after each row tile so the PE's colsum work overlaps ACT's exp.
        slab(abt, at_p, rs_aa_d, False, after_row=lambda rt: cs_group(rt))
        slab(bbt, bt_p, rs_bb_d, False, after_row=lambda rt: cs_group(RT + rt))
        nc.sync.dma_start(cs_ab_d[:], cs_sb[:])

    nc.compile()
    _PROGRAM = nc
    return nc


def _normalize(x):
    n = np.linalg.norm(x, axis=1, keepdims=True)
    return x / np.maximum(n, EPS)


def kernel(h1: np.ndarray, h2: np.ndarray):
    h1 = np.asarray(h1, dtype=np.float32)
    h2 = np.asarray(h2, dtype=np.float32)
    assert h1.shape == (N, D) and h2.shape == (N, D)

    a = _normalize(h1)
    b = _normalize(h2)
    diag = np.einsum("ij,ij->i", a, b, dtype=np.float64)

    bf = ml_dtypes.bfloat16
    at = np.ascontiguousarray(a.T).astype(bf)   # [128, 8192]
    bt = np.ascontiguousarray(b.T).astype(bf)

    _install_neff_disk_cache()
    nc = build_program()

    from concourse import bass_utils

    in_maps = []
    for c in range(NCORES):
        s = slice(c * BLK, (c + 1) * BLK)
        in_maps.append(
            {
                "at": at,
                "bt": bt,
                "abt": np.ascontiguousarray(at[:, s]),
                "bbt": np.ascontiguousarray(bt[:, s]),
            }
        )

    res = bass_utils.run_bass_kernel_spmd(nc, in_maps, core_ids=list(range(NCORES)))

    # Assemble per-row sums. rs tiles are [128 partitions, RT row-tiles]:
    # global row index = core*BLK + rt*128 + p  ->  rs.T.reshape(BLK).
    def rows(name):
        return np.concatenate(
            [res.results[c][name].astype(np.float64).T.reshape(BLK) for c in range(NCORES)]
        )

    rs_ab = rows("rs_ab")
    rs_aa = rows("rs_aa")
    rs_bb = rows("rs_bb")
    cs_ab = np.sum(
        [res.results[c]["cs_ab"][0].astype(np.float64) for c in range(NCORES)], axis=0
    )

    e2 = np.exp(2.0)
    denom1 = rs_aa - e2 + rs_ab
    denom2 = rs_bb - e2 + cs_ab
    l1 = np.log(denom1) - 2.0 * diag
    l2 = np.log(denom2) - 2.0 * diag
    loss = np.mean(0.5 * (l1 + l2))
    return (np.asarray(loss, dtype=np.float32), 1)


# revision 9
# speedup vs baseline: 1.4116x; 1.1236x over previous
"""GRACE contrastive loss on 8 Trainium2 NeuronCores (Bass/Tile).

loss = mean over i of 0.5*(l1_i + l2_i), where (T=0.5, a/b = row-normalized
h1/h2):
  l1_i = log(sum_j exp(a_i.a_j/T) - e^2 + sum_j exp(a_i.b_j/T)) - a_i.b_i/T
  l2_i = log(sum_j exp(b_i.b_j/T) - e^2 + sum_j exp(b_i.a_j/T)) - a_i.b_i/T

Work split over 8 cores, exploiting symmetry of the two reflexive
similarity matrices (only the upper/lower triangle of a@a.T / b@b.T is
exponentiated; the mirrored half is recovered from column sums):

- Phase B (all cores): rows c*1024..(c+1)*1024 of exp(a@b.T): matmul +
  exp with fused row-sum accumulation (ScalarE accum_out), exp values
  staged to SBUF in fp8 for column sums.
- Phase U (all cores): 9 "units" of 1024x1024. In the concatenated
  column-block space [a blocks 0-7 | b blocks 8-15], core c computes
  blocks c..c+8: that is rows a_c x upper-triangle columns of a, plus
  rows b_c x lower-triangle columns of b — a contiguous block run, so a
  single partition-id-derived register offset makes the program SPMD-
  uniform. Each unit emits row sums (accum_out) and column sums (VectorE
  tree-add over row tiles + ones-vector matmul partition reduce on PE).
- cs_ab groups: column sums of exp(a@b.T) via ones-matmuls over the fp8
  staging, PSUM-accumulated across row tiles, interleaved into phase U.

The host does the O(N*D) pieces: normalization, diag(a@b.T), final
assembly of row/column sums into the two denominators, log, mean.
"""

import hashlib
import os
from contextlib import ExitStack
from pathlib import Path

import ml_dtypes
import numpy as np

TEMPERATURE = 0.5
EPS = 1e-8
N, D = 8192, 128
NCORES = 8
BLK = N // NCORES          # 1024 rows per core / unit side
RT = BLK // 128            # 8 row tiles per block
NU = 9                     # units per core in phase U


def _install_neff_disk_cache():
    """Cache walrus NEFF compiles on disk so fresh-process runs are fast."""
    import concourse.bass2jax as bass2jax

    if getattr(bass2jax, "_grace_neff_cache", False):
        return
    inner = bass2jax.compile_bir_kernel
    cache_dir = Path(os.environ.get("XDG_CACHE_HOME", os.path.expanduser("~/.cache")))
    cache_dir = cache_dir / "bass_neff_cache"
    try:
        cache_dir.mkdir(parents=True, exist_ok=True)
    except OSError:
        return

    def cached(bir_json, tmpdir, neff_name="file.neff"):
        data = bir_json if isinstance(bir_json, bytes) else bir_json.encode()
        key = hashlib.sha256(data).hexdigest()
        path = cache_dir / f"{key}_{neff_name}"
        out_path = os.path.join(tmpdir, neff_name)
        if path.exists():
            with open(path, "rb") as f:
                neff = f.read()
            with open(out_path, "wb") as f:
                f.write(neff)
            return out_path
        res = inner(bir_json, tmpdir, neff_name)
        try:
            with open(res, "rb") as f:
                neff = f.read()
            tmp = path.with_suffix(".tmp%d" % os.getpid())
            with open(tmp, "wb") as f:
                f.write(neff)
            tmp.rename(path)
        except OSError:
            pass
        return res

    bass2jax.compile_bir_kernel = cached
    bass2jax._grace_neff_cache = True


_PROGRAM = None


def build_program():
    global _PROGRAM
    if _PROGRAM is not None:
        return _PROGRAM

    import concourse.bass as bass
    import concourse.tile as tile
    from concourse import bacc, mybir

    BF = mybir.dt.bfloat16
    F8 = mybir.dt.float8e4
    F32 = mybir.dt.float32
    Exp = mybir.ActivationFunctionType.Exp
    X = mybir.AxisListType.X

    nc = bacc.Bacc(
        "TRN2",
        target_bir_lowering=False,
        debug=False,
        enable_asserts=False,
        num_devices=NCORES,
    )
    at_d = nc.dram_tensor("at", [128, N], BF, kind="ExternalInput").ap()
    bt_d = nc.dram_tensor("bt", [128, N], BF, kind="ExternalInput").ap()
    abt_d = nc.dram_tensor("abt", [128, BLK], BF, kind="ExternalInput").ap()
    bbt_d = nc.dram_tensor("bbt", [128, BLK], BF, kind="ExternalInput").ap()
    rs_ab_d = nc.dram_tensor("rs_ab", [128, RT], F32, kind="ExternalOutput").ap()
    rs9_d = nc.dram_tensor("rs9", [128, NU * RT], F32, kind="ExternalOutput").ap()
    cs_ab_d = nc.dram_tensor("cs_ab", [1, N], BF, kind="ExternalOutput").ap()
    cs9_d = nc.dram_tensor("cs9", [1, NU * BLK], BF, kind="ExternalOutput").ap()

    with tile.TileContext(nc) as tc, ExitStack() as ctx:
        inp = ctx.enter_context(tc.tile_pool(name="inp", bufs=1))
        expp = ctx.enter_context(tc.tile_pool(name="expst", bufs=1))
        ustp = ctx.enter_context(tc.tile_pool(name="ust", bufs=2))
        lhsp = ctx.enter_context(tc.tile_pool(name="lhst", bufs=2))
        accp = ctx.enter_context(tc.tile_pool(name="acc", bufs=4))
        rsp = ctx.enter_context(tc.tile_pool(name="rs", bufs=1))
        csbp = ctx.enter_context(tc.tile_pool(name="csb", bufs=1))
        onep = ctx.enter_context(tc.tile_pool(name="ones", bufs=1))

        # ---- input DMAs (first-use order) ----
        PIECE = N // 4
        lhscat = inp.tile([128, 2 * BLK], BF)          # [abt | bbt]
        nc.sync.dma_start(lhscat[:, 0:BLK], abt_d[:])
        nc.sync.dma_start(lhscat[:, BLK : 2 * BLK], bbt_d[:])
        bt_p = []
        for i in range(4):
            t = inp.tile([128, PIECE], BF, tag=f"bt{i}")
            nc.sync.dma_start(t[:], bt_d[:, i * PIECE : (i + 1) * PIECE])
            bt_p.append(t)
        # concatenated [at | bt] column-block space for phase U
        atbt = inp.tile([128, 2 * N], BF)
        nc.sync.dma_start(atbt[:, 0:N], at_d[:])
        nc.sync.dma_start(atbt[:, N : 2 * N], bt_d[:])

        ones8 = onep.tile([128, 1], F8, tag="ones8")
        nc.vector.memset(ones8[:], 1.0)
        ones16 = onep.tile([128, 1], BF, tag="ones16")
        nc.vector.memset(ones16[:], 1.0)

        # fp8 staging of exp(a_blk@b^T) for the cs_ab column sums
        expst = expp.tile([128, RT * N], F8)
        cs_sb = csbp.tile([1, N], BF, tag="cs_sb")
        cs9_sb = csbp.tile([1, NU * BLK], BF, tag="cs9_sb")
        rs9_t = rsp.tile([128, NU * RT], F32, tag="rs9")

        pid = nc.partition_id()

        # ---- Phase B: between slab, full width, 2048-column ACT chunks ----
        with tc.tile_pool(name="mmB", bufs=2, space="PSUM") as mmB:
            rs_t = rsp.tile([128, RT], F32, tag="rs_ab")
            for rt in range(RT):
                lhsT = lhscat[:, rt * 128 : (rt + 1) * 128]
                acc = accp.tile([128, 4], F32)
                for ci in range(4):
                    mt = mmB.tile([128, 2048], F32)
                    for q in range(4):
                        off = ci * 2048 + q * 512
                        nc.tensor.matmul(
                            mt[:, q * 512 : (q + 1) * 512],
                            lhsT=lhsT,
                            rhs=bt_p[off // PIECE][:, off % PIECE : off % PIECE + 512],
                            start=True,
                            stop=True,
                        )
                    nc.scalar.activation(
                        expst[:, rt * N + ci * 2048 : rt * N + (ci + 1) * 2048],
                        mt[:, :],
                        Exp,
                        scale=2.0,
                        accum_out=acc[:, ci : ci + 1],
                    )
                nc.vector.reduce_sum(rs_t[:, rt : rt + 1], acc[:, :], axis=X)
            nc.sync.dma_start(rs_ab_d[:], rs_t[:])

        # ---- Phase U: 9 symmetric units + interleaved cs_ab groups ----
        with (
            tc.tile_pool(name="mmU", bufs=2, space="PSUM") as mmU,
            tc.tile_pool(name="ucs", bufs=2, space="PSUM") as ucs,
            tc.tile_pool(name="csp", bufs=2, space="PSUM") as csp,
        ):

            def csab_group(ct):
                cst = csp.tile([1, 512], F32)
                for rt in range(RT):
                    nc.tensor.matmul(
                        cst[:, :],
                        lhsT=ones8[:, :],
                        rhs=expst[:, rt * N + ct * 512 : rt * N + (ct + 1) * 512],
                        start=(rt == 0),
                        stop=(rt == RT - 1),
                    )
                nc.vector.tensor_copy(cs_sb[:, ct * 512 : (ct + 1) * 512], cst[:, :])

            csab_sched = iter(range(N // 512))
            for u in range(NU):
                # unit's column block in [at|bt] space: t = pid + u
                base = (pid + u) * BLK
                lhsoff = ((pid + u) & 8) * 128  # 0 -> abt half, 1024 -> bbt half
                lhst = lhsp.tile([128, BLK], BF)
                nc.vector.tensor_copy(lhst[:, :], lhscat[:, bass.ds(lhsoff, BLK)])
                ust = ustp.tile([128, RT * BLK], BF)
                for rt in range(RT):
                    mt = mmU.tile([128, BLK], F32)
                    for q in range(2):
                        nc.tensor.matmul(
                            mt[:, q * 512 : (q + 1) * 512],
                            lhsT=lhst[:, rt * 128 : (rt + 1) * 128],
                            rhs=atbt[:, bass.ds(base + q * 512, 512)],
                            start=True,
                            stop=True,
                        )
                    nc.scalar.activation(
                        ust[:, rt * BLK : (rt + 1) * BLK],
                        mt[:, :],
                        Exp,
                        scale=2.0,
                        accum_out=rs9_t[:, u * RT + rt : u * RT + rt + 1],
                    )
                # column sums of this unit: tree-add the 8 row tiles on DVE,
                # then partition-reduce via ones-matmuls on PE.
                for dst, src in [(1, 0), (3, 2), (5, 4), (7, 6), (3, 1), (7, 5), (7, 3)]:
                    nc.vector.tensor_add(
                        ust[:, dst * BLK : (dst + 1) * BLK],
                        ust[:, dst * BLK : (dst + 1) * BLK],
                        ust[:, src * BLK : (src + 1) * BLK],
                    )
                for h in range(2):
                    uc = ucs.tile([1, 512], F32)
                    nc.tensor.matmul(
                        uc[:, :],
                        lhsT=ones16[:, :],
                        rhs=ust[:, 7 * BLK + h * 512 : 7 * BLK + (h + 1) * 512],
                        start=True,
                        stop=True,
                    )
                    nc.vector.tensor_copy(
                        cs9_sb[:, u * BLK + h * 512 : u * BLK + (h + 1) * 512], uc[:, :]
                    )
                # interleave ~2 cs_ab groups per unit
                for _ in range(2):
                    ct = next(csab_sched, None)
                    if ct is not None:
                        csab_group(ct)
            for ct in csab_sched:
                csab_group(ct)

        nc.sync.dma_start(rs9_d[:], rs9_t[:])
        nc.sync.dma_start(cs9_d[:], cs9_sb[:])
        nc.sync.dma_start(cs_ab_d[:], cs_sb[:])

    nc.compile()
    _PROGRAM = nc
    return nc


def _normalize(x):
    n = np.linalg.norm(x, axis=1, keepdims=True)
    return x / np.maximum(n, EPS)


def kernel(h1: np.ndarray, h2: np.ndarray):
    h1 = np.asarray(h1, dtype=np.float32)
    h2 = np.asarray(h2, dtype=np.float32)
    assert h1.shape == (N, D) and h2.shape == (N, D)

    a = _normalize(h1)
    b = _normalize(h2)
    diag = np.einsum("ij,ij->i", a, b, dtype=np.float64)

    bf = ml_dtypes.bfloat16
    at = np.ascontiguousarray(a.T).astype(bf)   # [128, 8192]
    bt = np.ascontiguousarray(b.T).astype(bf)

    _install_neff_disk_cache()
    nc = build_program()

    from concourse import bass_utils

    in_maps = []
    for c in range(NCORES):
        s = slice(c * BLK, (c + 1) * BLK)
        in_maps.append(
            {
                "at": at,
                "bt": bt,
                "abt": np.ascontiguousarray(at[:, s]),
                "bbt": np.ascontiguousarray(bt[:, s]),
            }
        )

    res = bass_utils.run_bass_kernel_spmd(nc, in_maps, core_ids=list(range(NCORES)))

    # ---- host assembly ----
    # row-tile layout [128, RT] -> rows: global row = rt*128 + p
    def rows_of(arr):  # [128, k*RT] -> [k, BLK]
        k = arr.shape[1] // RT
        return arr.astype(np.float64).T.reshape(k, RT, 128).reshape(k, BLK)

    e2 = np.exp(2.0)
    rs_ab = np.concatenate([rows_of(res.results[c]["rs_ab"])[0] for c in range(NCORES)])
    cs_ab = np.sum(
        [res.results[c]["cs_ab"][0].astype(np.float64) for c in range(NCORES)], axis=0
    )

    rs_aa = np.zeros(N, dtype=np.float64)
    rs_bb = np.zeros(N, dtype=np.float64)
    for c in range(NCORES):
        rs9 = rows_of(res.results[c]["rs9"])          # [NU, BLK] row sums per unit
        cs9 = res.results[c]["cs9"][0].astype(np.float64)  # [NU*BLK] col sums per unit
        for u in range(NU):
            t = c + u  # column block in [a 0-7 | b 8-15] space
            if t < NCORES:
                # unit of a@a.T: rows block c, columns block t (t >= c)
                rs_aa[c * BLK : (c + 1) * BLK] += rs9[u]
                if u > 0:  # mirrored half: contributes to rows block t
                    rs_aa[t * BLK : (t + 1) * BLK] += cs9[u * BLK : (u + 1) * BLK]
            else:
                # unit of b@b.T: rows block c, columns block v (v <= c)
                v = t - NCORES
                rs_bb[c * BLK : (c + 1) * BLK] += rs9[u]
                if v < c:  # mirrored half: contributes to rows block v
                    rs_bb[v * BLK : (v + 1) * BLK] += cs9[u * BLK : (u + 1) * BLK]

    denom1 = rs_aa - e2 + rs_ab
    denom2 = rs_bb - e2 + cs_ab
    l1 = np.log(denom1) - 2.0 * diag
    l2 = np.log(denom2) - 2.0 * diag
    loss = np.mean(0.5 * (l1 + l2))
    return (np.asarray(loss, dtype=np.float32), 1)


# revision 13
# speedup vs baseline: 1.6527x; 1.1708x over previous
"""GRACE contrastive loss on 8 Trainium2 NeuronCores (Bass/Tile).

loss = mean over i of 0.5*(l1_i + l2_i), where (T=0.5, a/b = row-normalized
h1/h2):
  l1_i = log(sum_j exp(a_i.a_j/T) - e^2 + sum_j exp(a_i.b_j/T)) - a_i.b_i/T
  l2_i = log(sum_j exp(b_i.b_j/T) - e^2 + sum_j exp(b_i.a_j/T)) - a_i.b_i/T

Work split over 8 cores, exploiting symmetry of the two reflexive
similarity matrices (only the upper/lower triangle of a@a.T / b@b.T is
exponentiated; the mirrored half is recovered from column sums):

- Phase B (all cores): rows c*1024..(c+1)*1024 of exp(a@b.T): matmul +
  exp with fused row-sum accumulation (ScalarE accum_out), exp values
  staged to SBUF in fp8 for column sums.
- Phase U (all cores): 9 "units" of 1024x1024. In the concatenated
  column-block space [a blocks 0-7 | b blocks 8-15], core c computes
  blocks c..c+8: that is rows a_c x upper-triangle columns of a, plus
  rows b_c x lower-triangle columns of b — a contiguous block run, so a
  single partition-id-derived register offset makes the program SPMD-
  uniform. Each unit emits row sums (accum_out) and column sums (VectorE
  tree-add over row tiles + ones-vector matmul partition reduce on PE).
- cs_ab groups: column sums of exp(a@b.T) via ones-matmuls over the fp8
  staging, PSUM-accumulated across row tiles, interleaved into phase U.

The host does the O(N*D) pieces: normalization, diag(a@b.T), final
assembly of row/column sums into the two denominators, log, mean.
"""

import hashlib
import os
from contextlib import ExitStack
from pathlib import Path

import ml_dtypes
import numpy as np

TEMPERATURE = 0.5
EPS = 1e-8
N, D = 8192, 128
NCORES = 8
BLK = N // NCORES          # 1024 rows per core / unit side
RT = BLK // 128            # 8 row tiles per block
NU = 9                     # units per core in phase U


def _install_neff_disk_cache():
    """Cache walrus NEFF compiles on disk so fresh-process runs are fast."""
    import concourse.bass2jax as bass2jax

    if getattr(bass2jax, "_grace_neff_cache", False):
        return
    inner = bass2jax.compile_bir_kernel
    cache_dir = Path(os.environ.get("XDG_CACHE_HOME", os.path.expanduser("~/.cache")))
    cache_dir = cache_dir / "bass_neff_cache"
    try:
        cache_dir.mkdir(parents=True, exist_ok=True)
    except OSError:
        return

    def cached(bir_json, tmpdir, neff_name="file.neff"):
        data = bir_json if isinstance(bir_json, bytes) else bir_json.encode()
        key = hashlib.sha256(data).hexdigest()
        path = cache_dir / f"{key}_{neff_name}"
        out_path = os.path.join(tmpdir, neff_name)
        if path.exists():
            with open(path, "rb") as f:
                neff = f.read()
            with open(out_path, "wb") as f:
                f.write(neff)
            return out_path
        res = inner(bir_json, tmpdir, neff_name)
        try:
            with open(res, "rb") as f:
                neff = f.read()
            tmp = path.with_suffix(".tmp%d" % os.getpid())
            with open(tmp, "wb") as f:
                f.write(neff)
            tmp.rename(path)
        except OSError:
            pass
        return res

    bass2jax.compile_bir_kernel = cached
    bass2jax._grace_neff_cache = True


_PROGRAM = None


def build_program():
    global _PROGRAM
    if _PROGRAM is not None:
        return _PROGRAM

    import concourse.bass as bass
    import concourse.tile as tile
    from concourse import bacc, mybir

    BF = mybir.dt.bfloat16
    F8 = mybir.dt.float8e4
    F32 = mybir.dt.float32
    Exp = mybir.ActivationFunctionType.Exp
    X = mybir.AxisListType.X

    nc = bacc.Bacc(
        "TRN2",
        target_bir_lowering=False,
        debug=False,
        enable_asserts=False,
        num_devices=NCORES,
    )
    at_d = nc.dram_tensor("at", [128, N], BF, kind="ExternalInput").ap()
    bt_d = nc.dram_tensor("bt", [128, N], BF, kind="ExternalInput").ap()
    rs_ab_d = nc.dram_tensor("rs_ab", [128, RT], F32, kind="ExternalOutput").ap()
    rs9_d = nc.dram_tensor("rs9", [128, NU * RT], F32, kind="ExternalOutput").ap()
    cs_ab_d = nc.dram_tensor("cs_ab", [1, N], BF, kind="ExternalOutput").ap()
    cs9_d = nc.dram_tensor("cs9", [1, NU * BLK], BF, kind="ExternalOutput").ap()

    with tile.TileContext(nc) as tc, ExitStack() as ctx:
        inp = ctx.enter_context(tc.tile_pool(name="inp", bufs=1))
        expp = ctx.enter_context(tc.tile_pool(name="expst", bufs=1))
        ustp = ctx.enter_context(tc.tile_pool(name="ust", bufs=2))
        lhsp = ctx.enter_context(tc.tile_pool(name="lhst", bufs=2))
        accp = ctx.enter_context(tc.tile_pool(name="acc", bufs=4))
        rsp = ctx.enter_context(tc.tile_pool(name="rs", bufs=1))
        csbp = ctx.enter_context(tc.tile_pool(name="csb", bufs=1))
        onep = ctx.enter_context(tc.tile_pool(name="ones", bufs=1))

        # ---- input DMAs (first-use order) ----
        pid0 = nc.partition_id()
        PIECE = N // 4
        # this core's row blocks, sliced out of the full at/bt by partition id
        lhscat = inp.tile([128, 2 * BLK], BF)          # [a_blk | b_blk] transposed
        nc.sync.dma_start(lhscat[:, 0:BLK], at_d[:, bass.ds(pid0 * BLK, BLK)])
        nc.sync.dma_start(lhscat[:, BLK : 2 * BLK], bt_d[:, bass.ds(pid0 * BLK, BLK)])
        bt_p = []
        for i in range(4):
            t = inp.tile([128, PIECE], BF, tag=f"bt{i}")
            nc.sync.dma_start(t[:], bt_d[:, i * PIECE : (i + 1) * PIECE])
            bt_p.append(t)
        # concatenated [at | bt] column-block space for phase U
        atbt = inp.tile([128, 2 * N], BF)
        nc.sync.dma_start(atbt[:, 0:N], at_d[:])
        nc.sync.dma_start(atbt[:, N : 2 * N], bt_d[:])

        ones8 = onep.tile([128, 1], F8, tag="ones8")
        nc.vector.memset(ones8[:], 1.0)
        ones16 = onep.tile([128, 1], BF, tag="ones16")
        nc.vector.memset(ones16[:], 1.0)

        # fp8 staging of exp(a_blk@b^T) for the cs_ab column sums
        expst = expp.tile([128, RT * N], F8)
        cs_sb = csbp.tile([1, N], BF, tag="cs_sb")
        cs9_sb = csbp.tile([1, NU * BLK], BF, tag="cs9_sb")
        rs9_t = rsp.tile([128, NU * RT], F32, tag="rs9")

        pid = pid0

        # ---- Phase B: between slab, full width, 2048-column ACT chunks ----
        with tc.tile_pool(name="mmB", bufs=2, space="PSUM") as mmB:
            rs_t = rsp.tile([128, RT], F32, tag="rs_ab")
            for rt in range(RT):
                lhsT = lhscat[:, rt * 128 : (rt + 1) * 128]
                acc = accp.tile([128, 4], F32)
                for ci in range(4):
                    mt = mmB.tile([128, 2048], F32)
                    for q in range(4):
                        off = ci * 2048 + q * 512
                        nc.tensor.matmul(
                            mt[:, q * 512 : (q + 1) * 512],
                            lhsT=lhsT,
                            rhs=bt_p[off // PIECE][:, off % PIECE : off % PIECE + 512],
                            start=True,
                            stop=True,
                        )
                    nc.scalar.activation(
                        expst[:, rt * N + ci * 2048 : rt * N + (ci + 1) * 2048],
                        mt[:, :],
                        Exp,
                        scale=2.0,
                        accum_out=acc[:, ci : ci + 1],
                    )
                nc.vector.reduce_sum(rs_t[:, rt : rt + 1], acc[:, :], axis=X)
            nc.sync.dma_start(rs_ab_d[:], rs_t[:])

        # ---- Phase U: 9 symmetric units + interleaved cs_ab groups ----
        with (
            tc.tile_pool(name="mmU", bufs=2, space="PSUM") as mmU,
            tc.tile_pool(name="ucs", bufs=2, space="PSUM") as ucs,
            tc.tile_pool(name="csp", bufs=2, space="PSUM") as csp,
        ):

            def csab_group(ct):
                cst = csp.tile([1, 512], F32)
                for rt in range(RT):
                    nc.tensor.matmul(
                        cst[:, :],
                        lhsT=ones8[:, :],
                        rhs=expst[:, rt * N + ct * 512 : rt * N + (ct + 1) * 512],
                        start=(rt == 0),
                        stop=(rt == RT - 1),
                    )
                nc.vector.tensor_copy(cs_sb[:, ct * 512 : (ct + 1) * 512], cst[:, :])

            csab_sched = iter(range(N // 512))
            for u in range(NU):
                # unit's column block in [at|bt] space: t = pid + u
                base = (pid + u) * BLK
                lhsoff = ((pid + u) & 8) * 128  # 0 -> abt half, 1024 -> bbt half
                lhst = lhsp.tile([128, BLK], BF)
                nc.vector.tensor_copy(lhst[:, :], lhscat[:, bass.ds(lhsoff, BLK)])
                ust = ustp.tile([128, RT * BLK], BF)
                for rt in range(RT):
                    mt = mmU.tile([128, BLK], F32)
                    for q in range(2):
                        nc.tensor.matmul(
                            mt[:, q * 512 : (q + 1) * 512],
                            lhsT=lhst[:, rt * 128 : (rt + 1) * 128],
                            rhs=atbt[:, bass.ds(base + q * 512, 512)],
                            start=True,
                            stop=True,
                        )
                    nc.scalar.activation(
                        ust[:, rt * BLK : (rt + 1) * BLK],
                        mt[:, :],
                        Exp,
                        scale=2.0,
                        accum_out=rs9_t[:, u * RT + rt : u * RT + rt + 1],
                    )
                # column sums of this unit: tree-add the 8 row tiles on DVE,
                # then partition-reduce via ones-matmuls on PE.
                for dst, src in [(1, 0), (3, 2), (5, 4), (7, 6), (3, 1), (7, 5), (7, 3)]:
                    nc.vector.tensor_add(
                        ust[:, dst * BLK : (dst + 1) * BLK],
                        ust[:, dst * BLK : (dst + 1) * BLK],
                        ust[:, src * BLK : (src + 1) * BLK],
                    )
                for h in range(2):
                    uc = ucs.tile([1, 512], F32)
                    nc.tensor.matmul(
                        uc[:, :],
                        lhsT=ones16[:, :],
                        rhs=ust[:, 7 * BLK + h * 512 : 7 * BLK + (h + 1) * 512],
                        start=True,
                        stop=True,
                    )
                    nc.vector.tensor_copy(
                        cs9_sb[:, u * BLK + h * 512 : u * BLK + (h + 1) * 512], uc[:, :]
                    )
                # interleave ~2 cs_ab groups per unit
                for _ in range(2):
                    ct = next(csab_sched, None)
                    if ct is not None:
                        csab_group(ct)
            for ct in csab_sched:
                csab_group(ct)

        nc.sync.dma_start(rs9_d[:], rs9_t[:])
        nc.sync.dma_start(cs9_d[:], cs9_sb[:])
        nc.sync.dma_start(cs_ab_d[:], cs_sb[:])

    nc.compile()
    _PROGRAM = nc
    return nc


def _normalize(x):
    n = np.linalg.norm(x, axis=1, keepdims=True)
    return x / np.maximum(n, EPS)


def kernel(h1: np.ndarray, h2: np.ndarray):
    h1 = np.asarray(h1, dtype=np.float32)
    h2 = np.asarray(h2, dtype=np.float32)
    assert h1.shape == (N, D) and h2.shape == (N, D)

    a = _normalize(h1)
    b = _normalize(h2)
    diag = np.einsum("ij,ij->i", a, b, dtype=np.float64)

    bf = ml_dtypes.bfloat16
    at = np.ascontiguousarray(a.T).astype(bf)   # [128, 8192]
    bt = np.ascontiguousarray(b.T).astype(bf)

    _install_neff_disk_cache()
    nc = build_program()

    from concourse import bass_utils

    in_maps = [{"at": at, "bt": bt} for _ in range(NCORES)]
    res = bass_utils.run_bass_kernel_spmd(nc, in_maps, core_ids=list(range(NCORES)))

    # ---- host assembly ----
    # row-tile layout [128, RT] -> rows: global row = rt*128 + p
    def rows_of(arr):  # [128, k*RT] -> [k, BLK]
        k = arr.shape[1] // RT
        return arr.astype(np.float64).T.reshape(k, RT, 128).reshape(k, BLK)

    e2 = np.exp(2.0)
    rs_ab = np.concatenate([rows_of(res.results[c]["rs_ab"])[0] for c in range(NCORES)])
    cs_ab = np.sum(
        [res.results[c]["cs_ab"][0].astype(np.float64) for c in range(NCORES)], axis=0
    )

    rs_aa = np.zeros(N, dtype=np.float64)
    rs_bb = np.zeros(N, dtype=np.float64)
    for c in range(NCORES):
        rs9 = rows_of(res.results[c]["rs9"])          # [NU, BLK] row sums per unit
        cs9 = res.results[c]["cs9"][0].astype(np.float64)  # [NU*BLK] col sums per unit
        for u in range(NU):
            t = c + u  # column block in [a 0-7 | b 8-15] space
            if t < NCORES:
                # unit of a@a.T: rows block c, columns block t (t >= c)
                rs_aa[c * BLK : (c + 1) * BLK] += rs9[u]
                if u > 0:  # mirrored half: contributes to rows block t
                    rs_aa[t * BLK : (t + 1) * BLK] += cs9[u * BLK : (u + 1) * BLK]
            else:
                # unit of b@b.T: rows block c, columns block v (v <= c)
                v = t - NCORES
                rs_bb[c * BLK : (c + 1) * BLK] += rs9[u]
                if v < c:  # mirrored half: contributes to rows block v
                    rs_bb[v * BLK : (v + 1) * BLK] += cs9[u * BLK : (u + 1) * BLK]

    denom1 = rs_aa - e2 + rs_ab
    denom2 = rs_bb - e2 + cs_ab
    l1 = np.log(denom1) - 2.0 * diag
    l2 = np.log(denom2) - 2.0 * diag
    loss = np.mean(0.5 * (l1 + l2))
    return (np.asarray(loss, dtype=np.float32), 1)
